# revision 36
# baseline (speedup 1.0000x reference)
"""Trainium2 Bass kernel for nn_CPADConvOffsetStage.

The reference module is:
  up_posi = grid_sample_bilinear_border(posi_map -> [B,16,GP,GP], grid = base + offset*scale)
  h       = relu(w1 @ up_posi + b1)           (1x1 conv)
  weights = (w2 @ h + b2).reshape(B,64,9,H,W) (1x1 conv -> per-pixel 3x3 kernels)
  x_adapt = w_ca @ x                          (1x1 conv)
  out     = sum_k weights[:,:,k] * unfold3x3(x_adapt)[:,:,k] + bias

In setup_inputs() posi_map is spatially constant per channel (jnp.ones).
Bilinear interpolation weights sum to exactly 1, so up_posi is spatially
constant => h, weights are spatially constant => the whole module reduces
to ONE dense 3x3 convolution with host-precomputable weights
    Wfull[o,c,k] = wk[o,k] * w_ca[o,c],   wk = (w2 @ relu(w1 @ v + b1) + b2)
plus the bias.  The kernel below runs that conv data-parallel over batch
(1 batch image per NeuronCore, 8 cores).

If posi_map is NOT per-channel spatially constant (never the case for the
shipped setup_inputs), we fall back to an exact numpy port of the
reference.
"""

import os
import numpy as np
from contextlib import ExitStack

import concourse.bass as bass
import concourse.tile as tile
from concourse import mybir
from concourse.bass_utils import run_bass_kernel_spmd

# Problem constants (hardcoded per contract)
B, C, H, W = 8, 64, 128, 128
OC = 64
KK = 3
POSI_CH, GP = 16, 16
NCORES = 8
F32 = mybir.dt.float32

HPAD, WPAD = H + 2, W + 2      # host-padded image (130 x 130)
ROWS_PER_TILE = 4              # 4 rows * 128 cols = 512 = max fp32 moving free dim
NFREE = ROWS_PER_TILE * W      # 512
RB = 16                        # output rows per SBUF block
NBLK = H // RB                 # 8 blocks
SUB = RB // ROWS_PER_TILE      # 4 psum tiles per block
NXB = 4                        # rotated input block buffers
F32R = mybir.dt.float32r       # fp32 storage, single-pass relaxed-precision matmul

_cached_nc = None
_cached_variant = None
last_results = None            # test harness introspection


def _ensure_ntff_hook():
    """Register the axon NTFF-profile hook that this image's antenv lacks.

    run_bass_kernel_spmd(trace=True) under axon needs
    antenv.axon_hooks.get_axon_ntff_profile_hook; the hook machinery
    exists in trn_agent_boot but was never registered because
    antenv.axon_hooks is missing.  Recreate the module in sys.modules.
    """
    import sys
    import types

    if "antenv.axon_hooks" in sys.modules:
        return
    try:
        from trn_agent_boot.trn_boot import _ntff_profile_via_ctypes

        hook = _ntff_profile_via_ctypes("/opt/axon/libaxon_pjrt.so")
    except Exception:
        hook = None
    mod = types.ModuleType("antenv.axon_hooks")
    mod.get_axon_ntff_profile_hook = lambda: hook
    mod.set_axon_ntff_profile_hook = lambda h: None
    sys.modules["antenv.axon_hooks"] = mod
    try:
        import antenv

        antenv.axon_hooks = mod
    except Exception:
        pass


def _build_conv_nc(variant="bf16_tiled", split_waits=True):
    """3x3 conv, 64->64 ch, on one (host-padded) [64,130,130] image.

    SPMD over 8 cores, one batch image per core.  The input arrives
    zero-padded from the host so every 3x3 tap is a plain shifted
    window read; no memsets / halo special cases on device (this also
    keeps every Matmult at <=2 semaphore waits — walrus rejects 3+ on
    the LDWEIGHTS struct).
    """
    tiled = variant in ("bf16_tiled", "bf16_dup3")
    dup = variant == "bf16_dup3"
    mm_dt = mybir.dt.bfloat16 if tiled else F32R
    xb_parts = 128 if dup else C
    w_cols = 6 * OC if dup else 9 * OC
    nc = bass.Bass()
    x_d = nc.declare_dram_parameter("x", [C, HPAD, WPAD], mm_dt, isOutput=False)
    w_d = nc.declare_dram_parameter("wts", [xb_parts, w_cols], mm_dt, isOutput=False)
    b_d = nc.declare_dram_parameter("wb", [OC, 1], F32, isOutput=False)
    o_d = nc.declare_dram_parameter("out", [OC, H, W], F32, isOutput=True)

    with ExitStack() as ctx:
        tc = ctx.enter_context(tile.TileContext(nc))
        singles = ctx.enter_context(tc.tile_pool(name="singles", bufs=1))
        outs = ctx.enter_context(tc.tile_pool(name="outs", bufs=4))
        psum = ctx.enter_context(
            tc.tile_pool(name="psum", bufs=8 if dup else 4, space="PSUM")
        )
        tmps = ctx.enter_context(tc.tile_pool(name="tmps", bufs=4))

        w_sb = singles.tile([xb_parts, w_cols], mm_dt)
        nc.sync.dma_start(out=w_sb[:, :], in_=w_d[:, :])
        b_sb = singles.tile([OC, 1], F32)
        nc.sync.dma_start(out=b_sb[:, :], in_=b_d[:, :])

        # Rotated input blocks: 18 padded rows each (16 output rows + halo)
        xbs = []
        for i in range(NXB):
            xb_buf = singles.tile([xb_parts, RB + 2, WPAD], mm_dt, tag=f"xb{i}", name=f"xb{i}")
            if dup:
                # The duplicate (shifted) half never receives its last
                # column from DMA; zero it once so K=128 solo matmuls
                # (whose bottom-half weights are zero) can't hit NaN*0.
                nc.vector.memset(xb_buf[C : 2 * C, :, WPAD - 1 : WPAD], 0.0)
            xbs.append(xb_buf)

        # Tap k -> column group: A (psum partitions 0:64, tile_position (0,0))
        # gets taps 0,2,4,6,8; B (64:128, (0,64)) gets 1,3,5,7.  The two
        # column groups of the 128x128 PE array run concurrently (separate
        # XBUS streams), so 9 taps cost ~5 matmul slots instead of 9.
        for blk in range(NBLK):
            xb = xbs[blk % NXB]
            nc.sync.dma_start(
                out=xb[0:C, :, :],
                in_=x_d[:, blk * RB : blk * RB + RB + 2, :],
            )
            if dup:
                # partitions 64:128 = same rows shifted one column left, so a
                # K=128 matmul contracts two horizontally-adjacent taps at
                # once.  Derived on-chip (cross-partition copy) instead of a
                # second DRAM read — the DMA system is the bottleneck.
                nc.gpsimd.tensor_copy(
                    xb[C : 2 * C, :, 0 : WPAD - 1],
                    xb[0:C, :, 1:WPAD],
                )
            o_blk = outs.tile([OC, RB * W], F32)
            for s in range(SUB):
                r0 = s * ROWS_PER_TILE        # row offset within block
                if dup:
                    ps = psum.tile([128, NFREE], F32)
                    # 6 all-K=128 matmuls -> 3 col-tiled slots (K=128 streams
                    # ~2x faster per column than K=64 on this part, and the
                    # solo taps' bottom-half weights are zero):
                    #  A: pair(0,1) | pair(6,7) | solo k5
                    #  B: pair(3,4) | solo k2   | solo k8
                    mms = [
                        (0, 0, 0, True, False),   # pair row 0 -> A
                        (1, 1, 1, True, False),   # pair row 1 -> B
                        (2, 2, 0, False, False),  # pair row 2 -> A
                        (3, 0, 1, False, False),  # k2 (row 0, col 2) -> B
                        (4, 1, 0, False, True),   # k5 (row 1, col 2) -> A
                        (5, 2, 1, False, True),   # k8 (row 2, col 2) -> B
                    ]
                    for wi, i, col, st, sp in mms:
                        j = 0 if wi < 3 else 2
                        rhs = xb[:, r0 + i : r0 + i + ROWS_PER_TILE, j : j + W]
                        nc.tensor.matmul(
                            ps[64 * col : 64 * col + OC, :],
                            lhsT=w_sb[:, wi * OC : (wi + 1) * OC],
                            rhs=rhs,
                            start=st,
                            stop=sp,
                            tile_position=(0, 64 * col),
                            skip_group_check=True,
                        )
                elif tiled:
                    ps = psum.tile([128, NFREE], F32)
                    for k in range(9):
                        i, j = divmod(k, 3)
                        rhs = xb[:, r0 + i : r0 + i + ROWS_PER_TILE, j : j + W]
                        col = k % 2           # even taps -> A, odd -> B
                        nc.tensor.matmul(
                            ps[64 * col : 64 * col + OC, :],
                            lhsT=w_sb[:, k * OC : (k + 1) * OC],
                            rhs=rhs,
                            start=(k < 2),
                            stop=(k >= 7),
                            tile_position=(0, 64 * col),
                            skip_group_check=True,
                        )
                if tiled:
                    # Fold: B half evicted by ACT (with bias), A half added
                    # by DVE (walrus allows only one PSUM input per DVE op).
                    ps_a = ps[0:OC, 0:NFREE]
                    ps_b = ps[64:128, 0:NFREE]
                    tmpb = tmps.tile([OC, NFREE], F32)
                    nc.scalar.activation(
                        out=tmpb[:, :],
                        in_=ps_b,
                        func=mybir.ActivationFunctionType.Identity,
                        bias=b_sb[:, 0:1],
                        scale=1.0,
                    )
                    nc.vector.tensor_add(
                        o_blk[:, s * NFREE : (s + 1) * NFREE],
                        ps_a,
                        tmpb[:, :],
                    )
                else:
                    ps = psum.tile([OC, NFREE], F32)
                    for k in range(9):
                        i, j = divmod(k, 3)
                        rhs = xb[:, r0 + i : r0 + i + ROWS_PER_TILE, j : j + W]
                        nc.tensor.matmul(
                            ps[:, :],
                            lhsT=w_sb[:, k * OC : (k + 1) * OC],
                            rhs=rhs,
                            start=(k == 0),
                            stop=(k == 8),
                        )
                    nc.scalar.activation(
                        out=o_blk[:, s * NFREE : (s + 1) * NFREE],
                        in_=ps[:, :],
                        func=mybir.ActivationFunctionType.Identity,
                        bias=b_sb[:, 0:1],
                        scale=1.0,
                    )
            # Two half-block output DMAs: the first half drains while the
            # second half is still being computed (shorter kernel tail).
            HRB = RB // 2
            for h in range(2):
                nc.sync.dma_start(
                    out=o_d[:, blk * RB + h * HRB : blk * RB + (h + 1) * HRB, :],
                    in_=o_blk[:, h * HRB * W : (h + 1) * HRB * W].rearrange(
                        "p (r w) -> p r w", r=HRB
                    ),
                )
    if split_waits:
        _split_sync_waits(nc)
    return nc


def _split_sync_waits(nc, limit=1):
    """Hoist extra sync waits onto injected wait-only EventSemaphore ops.

    The neuronxcc walrus used under axon rejects compute instructions
    carrying more than one sync wait ("Too many sync wait commands", e.g.
    S3_LW / S3D3_AC structs).  Tile's sem assignment emits up to ~3.
    For every instruction with >limit waits, keep the first `limit` and
    prepend one wait-only EventSemaphore per extra wait on the same
    engine (same program position => same semantics).
    """
    import copy as _copy

    f = nc.m.functions[0]
    template = None
    for blk in f.blocks:
        for inst in blk.instructions:
            if type(inst).__name__ == "InstEventSemaphore":
                template = inst
                break
        if template is not None:
            break
    if template is None:
        return
    n_split = 0
    for blk in f.blocks:
        new_list = []
        changed = False
        for inst in blk.instructions:
            si = getattr(inst, "sync_info", None)
            op = str(getattr(inst, "opcode", ""))
            waits = list(si.on_wait) if (si and si.on_wait) else []
            if len(waits) > limit:
                for w in waits[limit:]:
                    ev = _copy.deepcopy(template)
                    ev.name = f"waitsplit_{n_split}"
                    n_split += 1
                    ev.engine = inst.engine
                    ev.sync_info = mybir.SyncInfo(on_wait=[w], on_update=[])
                    new_list.append(ev)
                inst.sync_info = mybir.SyncInfo(
                    on_wait=waits[:limit], on_update=list(si.on_update or [])
                )
                changed = True
            new_list.append(inst)
        if changed:
            blk.instructions = new_list


def _host_conv_weights(posi_map, w1, b1, w2, b2, w_ca, bias):
    """Collapse the constant-posi_map weight generator on the host."""
    pm = np.asarray(posi_map, np.float64)[0]              # [16, GP, GP]
    vvec = pm.reshape(POSI_CH, -1)[:, 0]                  # per-channel constant
    h = np.maximum(np.asarray(w1, np.float64) @ vvec + np.asarray(b1, np.float64), 0.0)
    wvec = np.asarray(w2, np.float64) @ h + np.asarray(b2, np.float64)   # [576]
    wk = wvec.reshape(OC, 9)                              # [o, k]
    wca = np.asarray(w_ca, np.float64)                    # [o, c]
    wfull = wk[:, None, :] * wca[:, :, None]              # [o, c, k]
    wts = np.ascontiguousarray(
        wfull.transpose(1, 2, 0).reshape(C, 9 * OC).astype(np.float32)
    )                                                     # [c, k*OC + o]
    wb = np.ascontiguousarray(
        np.asarray(bias, np.float32).reshape(OC, 1)
    )
    return wts, wb


def _pack_dup3(wts):
    """Repack [C, 9*OC] tap-major lhsT into the dup3 layout [128, 6*OC].

    Columns 0:3*OC are K=128 pairs (taps (3p, 3p+1) stacked on the
    partition axis, matching the +1-column-shifted input duplicate);
    columns 3*OC:6*OC are the K=64 solo taps (3q+2), bottom half zero.
    """
    w3 = np.zeros((128, 6 * OC), np.float32)
    for p in range(3):
        w3[0:C, p * OC:(p + 1) * OC] = wts[:, (3 * p) * OC:(3 * p + 1) * OC]
        w3[C:2 * C, p * OC:(p + 1) * OC] = wts[:, (3 * p + 1) * OC:(3 * p + 2) * OC]
        w3[0:C, (3 + p) * OC:(4 + p) * OC] = wts[:, (3 * p + 2) * OC:(3 * p + 3) * OC]
    return w3


def _numpy_reference(x, offset, posi_map, w1, b1, w2, b2, w_ca, bias):
    """Exact numpy port of reference.py (general-input fallback)."""
    x = np.asarray(x, np.float32)
    offset = np.asarray(offset, np.float32)
    posi_map = np.asarray(posi_map, np.float32)
    w1 = np.asarray(w1, np.float32)
    b1 = np.asarray(b1, np.float32)
    w2 = np.asarray(w2, np.float32)
    b2 = np.asarray(b2, np.float32)
    w_ca = np.asarray(w_ca, np.float32)
    bias = np.asarray(bias, np.float32)

    Bq, _, Hq, Wq = x.shape
    dx = offset[:, 0] * (2.0 / max(Wq - 1, 1)) * 0.5
    dy = offset[:, 1] * (2.0 / max(Hq - 1, 1)) * 0.5
    ys = np.linspace(-1.0, 1.0, Hq, dtype=x.dtype)
    xs = np.linspace(-1.0, 1.0, Wq, dtype=x.dtype)
    gx = xs[None, None, :] + dx
    gy = ys[None, :, None] + dy
    img = np.broadcast_to(posi_map, (Bq, posi_map.shape[1], GP, GP))

    Hp = Wp = GP
    imgT = img.transpose(0, 2, 3, 1)                      # [B, Hp, Wp, C]
    ix = np.clip((gx + 1.0) * 0.5 * (Wp - 1), 0.0, Wp - 1)
    iy = np.clip((gy + 1.0) * 0.5 * (Hp - 1), 0.0, Hp - 1)
    x0 = np.floor(ix).astype(np.int32)
    y0 = np.floor(iy).astype(np.int32)
    x1 = np.minimum(x0 + 1, Wp - 1)
    y1 = np.minimum(y0 + 1, Hp - 1)
    wx = (ix - x0.astype(ix.dtype))[..., None]
    wy = (iy - y0.astype(iy.dtype))[..., None]
    bb = np.arange(Bq)[:, None, None]
    v00 = imgT[bb, y0, x0]
    v01 = imgT[bb, y0, x1]
    v10 = imgT[bb, y1, x0]
    v11 = imgT[bb, y1, x1]
    top = v00 * (1 - wx) + v01 * wx
    bot = v10 * (1 - wx) + v11 * wx
    up = (top * (1 - wy) + bot * wy).transpose(0, 3, 1, 2)  # [B, 16, H, W]

    h = np.maximum(np.einsum('oc,bchw->bohw', w1, up) + b1[None, :, None, None], 0.0)
    weights = np.einsum('oc,bchw->bohw', w2, h) + b2[None, :, None, None]
    weights = weights.reshape(Bq, OC, KK * KK, Hq, Wq)
    x_adapt = np.einsum('oc,bchw->bohw', w_ca, x)
    xp = np.pad(x_adapt, ((0, 0), (0, 0), (1, 1), (1, 1)))
    patches = np.stack(
        [xp[:, :, i:i + Hq, j:j + Wq] for i in range(KK) for j in range(KK)],
        axis=2,
    )
    out = (weights * patches).sum(axis=2) + bias
    return out.astype(np.float32)


def kernel(**inputs):
    global _cached_nc, last_results
    x = np.ascontiguousarray(np.asarray(inputs["x"], np.float32))
    posi_map = np.asarray(inputs["posi_map"], np.float32)

    per_ch = posi_map.reshape(posi_map.shape[0] * posi_map.shape[1], -1)
    if not np.all(per_ch == per_ch[:, :1]):
        # general (spatially varying posi_map) fallback: exact numpy port
        return _numpy_reference(**{k: inputs[k] for k in (
            "x", "offset", "posi_map", "w1", "b1", "w2", "b2", "w_ca", "bias")})

    wts, wb = _host_conv_weights(
        posi_map, inputs["w1"], inputs["b1"], inputs["w2"], inputs["b2"],
        inputs["w_ca"], inputs["bias"],
    )

    variant = os.environ.get("BASS_KERNEL_VARIANT", "bf16_tiled")
    global _cached_variant
    if _cached_nc is None or _cached_variant != variant:
        _cached_nc = _build_conv_nc(variant)
        _cached_variant = variant

    xpad = np.pad(x, ((0, 0), (0, 0), (1, 1), (1, 1)))
    if variant == "bf16_dup3":
        wts = _pack_dup3(wts)
    if variant in ("bf16_tiled", "bf16_dup3"):
        import ml_dtypes

        xpad = xpad.astype(ml_dtypes.bfloat16)
        wts = wts.astype(ml_dtypes.bfloat16)
    in_maps = [{"x": xpad[i], "wts": wts, "wb": wb} for i in range(NCORES)]
    trace = bool(int(os.environ.get("BASS_KERNEL_TRACE", "0")))
    if trace:
        _ensure_ntff_hook()
    res = run_bass_kernel_spmd(
        _cached_nc, in_maps, list(range(NCORES)), trace=trace
    )
    last_results = res
    out = np.stack([res.results[i]["out"] for i in range(NCORES)], axis=0)
    return out


# revision 39
# speedup vs baseline: 1.5181x; 1.5181x over previous
"""Trainium2 Bass kernel for nn_CPADConvOffsetStage.

The reference module is:
  up_posi = grid_sample_bilinear_border(posi_map -> [B,16,GP,GP], grid = base + offset*scale)
  h       = relu(w1 @ up_posi + b1)           (1x1 conv)
  weights = (w2 @ h + b2).reshape(B,64,9,H,W) (1x1 conv -> per-pixel 3x3 kernels)
  x_adapt = w_ca @ x                          (1x1 conv)
  out     = sum_k weights[:,:,k] * unfold3x3(x_adapt)[:,:,k] + bias

In setup_inputs() posi_map is spatially constant per channel (jnp.ones).
Bilinear interpolation weights sum to exactly 1, so up_posi is spatially
constant => h, weights are spatially constant => the whole module reduces
to ONE dense 3x3 convolution with host-precomputable weights
    Wfull[o,c,k] = wk[o,k] * w_ca[o,c],   wk = (w2 @ relu(w1 @ v + b1) + b2)
plus the bias.  The kernel below runs that conv data-parallel over batch
(1 batch image per NeuronCore, 8 cores).

If posi_map is NOT per-channel spatially constant (never the case for the
shipped setup_inputs), we fall back to an exact numpy port of the
reference.
"""

import os
import numpy as np
from contextlib import ExitStack

import concourse.bass as bass
import concourse.tile as tile
from concourse import mybir
from concourse.bass_utils import run_bass_kernel_spmd

# Problem constants (hardcoded per contract)
B, C, H, W = 8, 64, 128, 128
OC = 64
KK = 3
POSI_CH, GP = 16, 16
NCORES = 8
F32 = mybir.dt.float32

HPAD, WPAD = H + 2, W + 2      # host-padded image (130 x 130)
ROWS_PER_TILE = 4              # 4 rows * 128 cols = 512 = max fp32 moving free dim
NFREE = ROWS_PER_TILE * W      # 512
RB = 16                        # output rows per SBUF block
NBLK = H // RB                 # 8 blocks
SUB = RB // ROWS_PER_TILE      # 4 psum tiles per block
NXB = 4                        # rotated input block buffers
F32R = mybir.dt.float32r       # fp32 storage, single-pass relaxed-precision matmul

_cached_nc = None
_cached_variant = None
last_results = None            # test harness introspection


def _ensure_ntff_hook():
    """Register the axon NTFF-profile hook that this image's antenv lacks.

    run_bass_kernel_spmd(trace=True) under axon needs
    antenv.axon_hooks.get_axon_ntff_profile_hook; the hook machinery
    exists in trn_agent_boot but was never registered because
    antenv.axon_hooks is missing.  Recreate the module in sys.modules.
    """
    import sys
    import types

    if "antenv.axon_hooks" in sys.modules:
        return
    try:
        from trn_agent_boot.trn_boot import _ntff_profile_via_ctypes

        hook = _ntff_profile_via_ctypes("/opt/axon/libaxon_pjrt.so")
    except Exception:
        hook = None
    mod = types.ModuleType("antenv.axon_hooks")
    mod.get_axon_ntff_profile_hook = lambda: hook
    mod.set_axon_ntff_profile_hook = lambda h: None
    sys.modules["antenv.axon_hooks"] = mod
    try:
        import antenv

        antenv.axon_hooks = mod
    except Exception:
        pass


def _build_conv_nc(variant="bf16_tiled", split_waits=True):
    """3x3 conv, 64->64 ch, on one (host-padded) [64,130,130] image.

    SPMD over 8 cores, one batch image per core.  The input arrives
    zero-padded from the host so every 3x3 tap is a plain shifted
    window read; no memsets / halo special cases on device (this also
    keeps every Matmult at <=2 semaphore waits — walrus rejects 3+ on
    the LDWEIGHTS struct).
    """
    tiled = variant in ("bf16_tiled", "bf16_dup3")
    dup = variant == "bf16_dup3"
    mm_dt = mybir.dt.bfloat16 if tiled else F32R
    xb_parts = 128 if dup else C
    w_cols = 6 * OC if dup else 9 * OC
    nc = bass.Bass()
    x_d = nc.declare_dram_parameter("x", [C, HPAD, WPAD], mm_dt, isOutput=False)
    w_d = nc.declare_dram_parameter("wts", [xb_parts, w_cols], mm_dt, isOutput=False)
    b_d = nc.declare_dram_parameter("wb", [OC, 1], F32, isOutput=False)
    o_d = nc.declare_dram_parameter("out", [OC, H, W], F32, isOutput=True)

    with ExitStack() as ctx:
        tc = ctx.enter_context(tile.TileContext(nc))
        singles = ctx.enter_context(tc.tile_pool(name="singles", bufs=1))
        outs = ctx.enter_context(tc.tile_pool(name="outs", bufs=4))
        psum = ctx.enter_context(
            tc.tile_pool(name="psum", bufs=2 if dup else 4, space="PSUM")
        )
        tmps = ctx.enter_context(tc.tile_pool(name="tmps", bufs=4))

        w_sb = singles.tile([xb_parts, w_cols], mm_dt)
        nc.sync.dma_start(out=w_sb[:, :], in_=w_d[:, :])
        b_sb = singles.tile([OC, 1], F32)
        nc.sync.dma_start(out=b_sb[:, :], in_=b_d[:, :])

        # Rotated input blocks: 18 padded rows each (16 output rows + halo)
        xbs = []
        for i in range(NXB):
            xb_buf = singles.tile([xb_parts, RB + 2, WPAD], mm_dt, tag=f"xb{i}", name=f"xb{i}")
            if dup:
                # The duplicate (shifted) half never receives its last
                # column from DMA; zero it once so K=128 solo matmuls
                # (whose bottom-half weights are zero) can't hit NaN*0.
                nc.vector.memset(xb_buf[C : 2 * C, :, WPAD - 1 : WPAD], 0.0)
            xbs.append(xb_buf)

        # Tap k -> column group: A (psum partitions 0:64, tile_position (0,0))
        # gets taps 0,2,4,6,8; B (64:128, (0,64)) gets 1,3,5,7.  The two
        # column groups of the 128x128 PE array run concurrently (separate
        # XBUS streams), so 9 taps cost ~5 matmul slots instead of 9.
        for blk in range(NBLK):
            xb = xbs[blk % NXB]
            nc.sync.dma_start(
                out=xb[0:C, :, :],
                in_=x_d[:, blk * RB : blk * RB + RB + 2, :],
            )
            if dup:
                # partitions 64:128 = same rows shifted one column left, so a
                # K=128 matmul contracts two horizontally-adjacent taps at
                # once.  Derived on-chip (cross-partition copy) instead of a
                # second DRAM read — the DMA system is the bottleneck.
                nc.vector.tensor_copy(
                    xb[C : 2 * C, :, 0 : WPAD - 1],
                    xb[0:C, :, 1:WPAD],
                )
            o_blk = outs.tile([OC, RB * W], F32)
            if dup:
                # Two row-groups (8 output rows = 1024 px) per 4-bank psum
                # tile: col group A accumulates in banks 0-1, B in banks 2-3,
                # so the ACT (B) and DVE (A) fold reads hit different banks
                # (they can only access PSUM in parallel on different banks)
                # and the fold runs once per 1024 px (half the per-op
                # overhead and half the psum-release waits on the PE).
                for s2 in range(SUB // 2):
                    ps = psum.tile([128, 4 * NFREE], F32)
                    for sh in range(2):
                        r0 = (s2 * 2 + sh) * ROWS_PER_TILE
                        # 6 all-K=128 matmuls -> 3 col-tiled slots:
                        #  A: pair(0,1) | pair(6,7) | solo k5
                        #  B: pair(3,4) | solo k2   | solo k8
                        mms = [
                            (0, 0, 0, True, False),   # pair row 0 -> A
                            (1, 1, 1, True, False),   # pair row 1 -> B
                            (2, 2, 0, False, False),  # pair row 2 -> A
                            (3, 0, 1, False, False),  # k2 (row 0, col 2) -> B
                            (4, 1, 0, False, True),   # k5 (row 1, col 2) -> A
                            (5, 2, 1, False, True),   # k8 (row 2, col 2) -> B
                        ]
                        for wi, i, col, st, sp in mms:
                            j = 0 if wi < 3 else 2
                            rhs = xb[:, r0 + i : r0 + i + ROWS_PER_TILE, j : j + W]
                            base = (2 * col + sh) * NFREE  # bank 2*col+sh
                            nc.tensor.matmul(
                                ps[64 * col : 64 * col + OC,
                                   base : base + NFREE],
                                lhsT=w_sb[:, wi * OC : (wi + 1) * OC],
                                rhs=rhs,
                                start=st,
                                stop=sp,
                                tile_position=(0, 64 * col),
                                skip_group_check=True,
                            )
                    # Fold both halves at once: banks 2-3 (B) via ACT with
                    # bias, then DVE adds banks 0-1 (A).
                    tmpb = tmps.tile([OC, 2 * NFREE], F32)
                    nc.scalar.activation(
                        out=tmpb[:, :],
                        in_=ps[64:128, 2 * NFREE : 4 * NFREE],
                        func=mybir.ActivationFunctionType.Identity,
                        bias=b_sb[:, 0:1],
                        scale=1.0,
                    )
                    nc.vector.tensor_add(
                        o_blk[:, s2 * 2 * NFREE : (s2 + 1) * 2 * NFREE],
                        ps[0:OC, 0 : 2 * NFREE],
                        tmpb[:, :],
                    )
            for s in range(0 if dup else SUB):
                r0 = s * ROWS_PER_TILE        # row offset within block
                if tiled:
                    ps = psum.tile([128, NFREE], F32)
                    for k in range(9):
                        i, j = divmod(k, 3)
                        rhs = xb[:, r0 + i : r0 + i + ROWS_PER_TILE, j : j + W]
                        col = k % 2           # even taps -> A, odd -> B
                        nc.tensor.matmul(
                            ps[64 * col : 64 * col + OC, :],
                            lhsT=w_sb[:, k * OC : (k + 1) * OC],
                            rhs=rhs,
                            start=(k < 2),
                            stop=(k >= 7),
                            tile_position=(0, 64 * col),
                            skip_group_check=True,
                        )
                if tiled:
                    # Fold: B half evicted by ACT (with bias), A half added
                    # by DVE (walrus allows only one PSUM input per DVE op).
                    ps_a = ps[0:OC, 0:NFREE]
                    ps_b = ps[64:128, 0:NFREE]
                    tmpb = tmps.tile([OC, NFREE], F32)
                    nc.scalar.activation(
                        out=tmpb[:, :],
                        in_=ps_b,
                        func=mybir.ActivationFunctionType.Identity,
                        bias=b_sb[:, 0:1],
                        scale=1.0,
                    )
                    nc.vector.tensor_add(
                        o_blk[:, s * NFREE : (s + 1) * NFREE],
                        ps_a,
                        tmpb[:, :],
                    )
                else:
                    ps = psum.tile([OC, NFREE], F32)
                    for k in range(9):
                        i, j = divmod(k, 3)
                        rhs = xb[:, r0 + i : r0 + i + ROWS_PER_TILE, j : j + W]
                        nc.tensor.matmul(
                            ps[:, :],
                            lhsT=w_sb[:, k * OC : (k + 1) * OC],
                            rhs=rhs,
                            start=(k == 0),
                            stop=(k == 8),
                        )
                    nc.scalar.activation(
                        out=o_blk[:, s * NFREE : (s + 1) * NFREE],
                        in_=ps[:, :],
                        func=mybir.ActivationFunctionType.Identity,
                        bias=b_sb[:, 0:1],
                        scale=1.0,
                    )
            # Two half-block output DMAs: the first half drains while the
            # second half is still being computed (shorter kernel tail).
            HRB = RB // 2
            for h in range(2):
                nc.sync.dma_start(
                    out=o_d[:, blk * RB + h * HRB : blk * RB + (h + 1) * HRB, :],
                    in_=o_blk[:, h * HRB * W : (h + 1) * HRB * W].rearrange(
                        "p (r w) -> p r w", r=HRB
                    ),
                )
    if split_waits:
        _split_sync_waits(nc)
    return nc


def _split_sync_waits(nc, limit=1):
    """Hoist extra sync waits onto injected wait-only EventSemaphore ops.

    The neuronxcc walrus used under axon rejects compute instructions
    carrying more than one sync wait ("Too many sync wait commands", e.g.
    S3_LW / S3D3_AC structs).  Tile's sem assignment emits up to ~3.
    For every instruction with >limit waits, keep the first `limit` and
    prepend one wait-only EventSemaphore per extra wait on the same
    engine (same program position => same semantics).
    """
    import copy as _copy

    f = nc.m.functions[0]
    template = None
    for blk in f.blocks:
        for inst in blk.instructions:
            if type(inst).__name__ == "InstEventSemaphore":
                template = inst
                break
        if template is not None:
            break
    if template is None:
        return
    n_split = 0
    for blk in f.blocks:
        new_list = []
        changed = False
        for inst in blk.instructions:
            si = getattr(inst, "sync_info", None)
            op = str(getattr(inst, "opcode", ""))
            waits = list(si.on_wait) if (si and si.on_wait) else []
            if len(waits) > limit:
                for w in waits[limit:]:
                    ev = _copy.deepcopy(template)
                    ev.name = f"waitsplit_{n_split}"
                    n_split += 1
                    ev.engine = inst.engine
                    ev.sync_info = mybir.SyncInfo(on_wait=[w], on_update=[])
                    new_list.append(ev)
                inst.sync_info = mybir.SyncInfo(
                    on_wait=waits[:limit], on_update=list(si.on_update or [])
                )
                changed = True
            new_list.append(inst)
        if changed:
            blk.instructions = new_list


def _host_conv_weights(posi_map, w1, b1, w2, b2, w_ca, bias):
    """Collapse the constant-posi_map weight generator on the host."""
    pm = np.asarray(posi_map, np.float64)[0]              # [16, GP, GP]
    vvec = pm.reshape(POSI_CH, -1)[:, 0]                  # per-channel constant
    h = np.maximum(np.asarray(w1, np.float64) @ vvec + np.asarray(b1, np.float64), 0.0)
    wvec = np.asarray(w2, np.float64) @ h + np.asarray(b2, np.float64)   # [576]
    wk = wvec.reshape(OC, 9)                              # [o, k]
    wca = np.asarray(w_ca, np.float64)                    # [o, c]
    wfull = wk[:, None, :] * wca[:, :, None]              # [o, c, k]
    wts = np.ascontiguousarray(
        wfull.transpose(1, 2, 0).reshape(C, 9 * OC).astype(np.float32)
    )                                                     # [c, k*OC + o]
    wb = np.ascontiguousarray(
        np.asarray(bias, np.float32).reshape(OC, 1)
    )
    return wts, wb


def _pack_dup3(wts):
    """Repack [C, 9*OC] tap-major lhsT into the dup3 layout [128, 6*OC].

    Columns 0:3*OC are K=128 pairs (taps (3p, 3p+1) stacked on the
    partition axis, matching the +1-column-shifted input duplicate);
    columns 3*OC:6*OC are the K=64 solo taps (3q+2), bottom half zero.
    """
    w3 = np.zeros((128, 6 * OC), np.float32)
    for p in range(3):
        w3[0:C, p * OC:(p + 1) * OC] = wts[:, (3 * p) * OC:(3 * p + 1) * OC]
        w3[C:2 * C, p * OC:(p + 1) * OC] = wts[:, (3 * p + 1) * OC:(3 * p + 2) * OC]
        w3[0:C, (3 + p) * OC:(4 + p) * OC] = wts[:, (3 * p + 2) * OC:(3 * p + 3) * OC]
    return w3


def _numpy_reference(x, offset, posi_map, w1, b1, w2, b2, w_ca, bias):
    """Exact numpy port of reference.py (general-input fallback)."""
    x = np.asarray(x, np.float32)
    offset = np.asarray(offset, np.float32)
    posi_map = np.asarray(posi_map, np.float32)
    w1 = np.asarray(w1, np.float32)
    b1 = np.asarray(b1, np.float32)
    w2 = np.asarray(w2, np.float32)
    b2 = np.asarray(b2, np.float32)
    w_ca = np.asarray(w_ca, np.float32)
    bias = np.asarray(bias, np.float32)

    Bq, _, Hq, Wq = x.shape
    dx = offset[:, 0] * (2.0 / max(Wq - 1, 1)) * 0.5
    dy = offset[:, 1] * (2.0 / max(Hq - 1, 1)) * 0.5
    ys = np.linspace(-1.0, 1.0, Hq, dtype=x.dtype)
    xs = np.linspace(-1.0, 1.0, Wq, dtype=x.dtype)
    gx = xs[None, None, :] + dx
    gy = ys[None, :, None] + dy
    img = np.broadcast_to(posi_map, (Bq, posi_map.shape[1], GP, GP))

    Hp = Wp = GP
    imgT = img.transpose(0, 2, 3, 1)                      # [B, Hp, Wp, C]
    ix = np.clip((gx + 1.0) * 0.5 * (Wp - 1), 0.0, Wp - 1)
    iy = np.clip((gy + 1.0) * 0.5 * (Hp - 1), 0.0, Hp - 1)
    x0 = np.floor(ix).astype(np.int32)
    y0 = np.floor(iy).astype(np.int32)
    x1 = np.minimum(x0 + 1, Wp - 1)
    y1 = np.minimum(y0 + 1, Hp - 1)
    wx = (ix - x0.astype(ix.dtype))[..., None]
    wy = (iy - y0.astype(iy.dtype))[..., None]
    bb = np.arange(Bq)[:, None, None]
    v00 = imgT[bb, y0, x0]
    v01 = imgT[bb, y0, x1]
    v10 = imgT[bb, y1, x0]
    v11 = imgT[bb, y1, x1]
    top = v00 * (1 - wx) + v01 * wx
    bot = v10 * (1 - wx) + v11 * wx
    up = (top * (1 - wy) + bot * wy).transpose(0, 3, 1, 2)  # [B, 16, H, W]

    h = np.maximum(np.einsum('oc,bchw->bohw', w1, up) + b1[None, :, None, None], 0.0)
    weights = np.einsum('oc,bchw->bohw', w2, h) + b2[None, :, None, None]
    weights = weights.reshape(Bq, OC, KK * KK, Hq, Wq)
    x_adapt = np.einsum('oc,bchw->bohw', w_ca, x)
    xp = np.pad(x_adapt, ((0, 0), (0, 0), (1, 1), (1, 1)))
    patches = np.stack(
        [xp[:, :, i:i + Hq, j:j + Wq] for i in range(KK) for j in range(KK)],
        axis=2,
    )
    out = (weights * patches).sum(axis=2) + bias
    return out.astype(np.float32)


def kernel(**inputs):
    global _cached_nc, last_results
    x = np.ascontiguousarray(np.asarray(inputs["x"], np.float32))
    posi_map = np.asarray(inputs["posi_map"], np.float32)

    per_ch = posi_map.reshape(posi_map.shape[0] * posi_map.shape[1], -1)
    if not np.all(per_ch == per_ch[:, :1]):
        # general (spatially varying posi_map) fallback: exact numpy port
        return _numpy_reference(**{k: inputs[k] for k in (
            "x", "offset", "posi_map", "w1", "b1", "w2", "b2", "w_ca", "bias")})

    wts, wb = _host_conv_weights(
        posi_map, inputs["w1"], inputs["b1"], inputs["w2"], inputs["b2"],
        inputs["w_ca"], inputs["bias"],
    )

    variant = os.environ.get("BASS_KERNEL_VARIANT", "bf16_tiled")
    global _cached_variant
    if _cached_nc is None or _cached_variant != variant:
        _cached_nc = _build_conv_nc(variant)
        _cached_variant = variant

    xpad = np.pad(x, ((0, 0), (0, 0), (1, 1), (1, 1)))
    if variant == "bf16_dup3":
        wts = _pack_dup3(wts)
    if variant in ("bf16_tiled", "bf16_dup3"):
        import ml_dtypes

        xpad = xpad.astype(ml_dtypes.bfloat16)
        wts = wts.astype(ml_dtypes.bfloat16)
    in_maps = [{"x": xpad[i], "wts": wts, "wb": wb} for i in range(NCORES)]
    trace = bool(int(os.environ.get("BASS_KERNEL_TRACE", "0")))
    if trace:
        _ensure_ntff_hook()
    res = run_bass_kernel_spmd(
        _cached_nc, in_maps, list(range(NCORES)), trace=trace
    )
    last_results = res
    out = np.stack([res.results[i]["out"] for i in range(NCORES)], axis=0)
    return out


# revision 43
# speedup vs baseline: 1.7549x; 1.1560x over previous
"""Trainium2 Bass kernel for nn_CPADConvOffsetStage.

The reference module is:
  up_posi = grid_sample_bilinear_border(posi_map -> [B,16,GP,GP], grid = base + offset*scale)
  h       = relu(w1 @ up_posi + b1)           (1x1 conv)
  weights = (w2 @ h + b2).reshape(B,64,9,H,W) (1x1 conv -> per-pixel 3x3 kernels)
  x_adapt = w_ca @ x                          (1x1 conv)
  out     = sum_k weights[:,:,k] * unfold3x3(x_adapt)[:,:,k] + bias

In setup_inputs() posi_map is spatially constant per channel (jnp.ones).
Bilinear interpolation weights sum to exactly 1, so up_posi is spatially
constant => h, weights are spatially constant => the whole module reduces
to ONE dense 3x3 convolution with host-precomputable weights
    Wfull[o,c,k] = wk[o,k] * w_ca[o,c],   wk = (w2 @ relu(w1 @ v + b1) + b2)
plus the bias.  The kernel below runs that conv data-parallel over batch
(1 batch image per NeuronCore, 8 cores).

If posi_map is NOT per-channel spatially constant (never the case for the
shipped setup_inputs), we fall back to an exact numpy port of the
reference.
"""

import os
import numpy as np
from contextlib import ExitStack

import concourse.bass as bass
import concourse.tile as tile
from concourse import mybir
from concourse.bass_utils import run_bass_kernel_spmd

# Problem constants (hardcoded per contract)
B, C, H, W = 8, 64, 128, 128
OC = 64
KK = 3
POSI_CH, GP = 16, 16
NCORES = 8
F32 = mybir.dt.float32

HPAD, WPAD = H + 2, W + 2      # host-padded image (130 x 130)
ROWS_PER_TILE = 4              # 4 rows * 128 cols = 512 = max fp32 moving free dim
NFREE = ROWS_PER_TILE * W      # 512
RB = 16                        # output rows per SBUF block
NBLK = H // RB                 # 8 blocks
SUB = RB // ROWS_PER_TILE      # 4 psum tiles per block
NXB = 4                        # rotated input block buffers
F32R = mybir.dt.float32r       # fp32 storage, single-pass relaxed-precision matmul

_cached_nc = None
_cached_variant = None
last_results = None            # test harness introspection


def _ensure_ntff_hook():
    """Register the axon NTFF-profile hook that this image's antenv lacks.

    run_bass_kernel_spmd(trace=True) under axon needs
    antenv.axon_hooks.get_axon_ntff_profile_hook; the hook machinery
    exists in trn_agent_boot but was never registered because
    antenv.axon_hooks is missing.  Recreate the module in sys.modules.
    """
    import sys
    import types

    if "antenv.axon_hooks" in sys.modules:
        return
    try:
        from trn_agent_boot.trn_boot import _ntff_profile_via_ctypes

        hook = _ntff_profile_via_ctypes("/opt/axon/libaxon_pjrt.so")
    except Exception:
        hook = None
    mod = types.ModuleType("antenv.axon_hooks")
    mod.get_axon_ntff_profile_hook = lambda: hook
    mod.set_axon_ntff_profile_hook = lambda h: None
    sys.modules["antenv.axon_hooks"] = mod
    try:
        import antenv

        antenv.axon_hooks = mod
    except Exception:
        pass


def _build_conv_nc(variant="bf16_tiled", split_waits=True):
    """3x3 conv, 64->64 ch, on one (host-padded) [64,130,130] image.

    SPMD over 8 cores, one batch image per core.  The input arrives
    zero-padded from the host so every 3x3 tap is a plain shifted
    window read; no memsets / halo special cases on device (this also
    keeps every Matmult at <=2 semaphore waits — walrus rejects 3+ on
    the LDWEIGHTS struct).
    """
    o16 = variant.endswith("_o16")
    base_variant = variant[:-4] if o16 else variant
    tiled = base_variant in ("bf16_tiled", "bf16_dup3")
    dup = base_variant == "bf16_dup3"
    mm_dt = mybir.dt.bfloat16 if tiled else F32R
    o_dt = mybir.dt.bfloat16 if o16 else F32
    xb_parts = 128 if dup else C
    w_cols = 6 * OC if dup else 9 * OC
    nc = bass.Bass()
    x_d = nc.declare_dram_parameter("x", [C, HPAD, WPAD], mm_dt, isOutput=False)
    w_d = nc.declare_dram_parameter("wts", [xb_parts, w_cols], mm_dt, isOutput=False)
    b_d = nc.declare_dram_parameter("wb", [OC, 1], F32, isOutput=False)
    o_d = nc.declare_dram_parameter("out", [OC, H, W], o_dt, isOutput=True)

    with ExitStack() as ctx:
        tc = ctx.enter_context(tile.TileContext(nc))
        singles = ctx.enter_context(tc.tile_pool(name="singles", bufs=1))
        outs = ctx.enter_context(tc.tile_pool(name="outs", bufs=4))
        psum = ctx.enter_context(
            tc.tile_pool(name="psum", bufs=8 if dup else 4, space="PSUM")
        )
        tmps = ctx.enter_context(tc.tile_pool(name="tmps", bufs=4))

        w_sb = singles.tile([xb_parts, w_cols], mm_dt)
        nc.sync.dma_start(out=w_sb[:, :], in_=w_d[:, :])
        b_sb = singles.tile([OC, 1], F32)
        nc.sync.dma_start(out=b_sb[:, :], in_=b_d[:, :])

        # Rotated input blocks: 18 padded rows each (16 output rows + halo)
        xbs = []
        for i in range(NXB):
            xb_buf = singles.tile([xb_parts, RB + 2, WPAD], mm_dt, tag=f"xb{i}", name=f"xb{i}")
            if dup:
                # The duplicate (shifted) half never receives its last
                # column from DMA; zero it once so K=128 solo matmuls
                # (whose bottom-half weights are zero) can't hit NaN*0.
                nc.vector.memset(xb_buf[C : 2 * C, :, WPAD - 1 : WPAD], 0.0)
            xbs.append(xb_buf)

        # Tap k -> column group: A (psum partitions 0:64, tile_position (0,0))
        # gets taps 0,2,4,6,8; B (64:128, (0,64)) gets 1,3,5,7.  The two
        # column groups of the 128x128 PE array run concurrently (separate
        # XBUS streams), so 9 taps cost ~5 matmul slots instead of 9.
        for blk in range(NBLK):
            xb = xbs[blk % NXB]
            nc.sync.dma_start(
                out=xb[0:C, :, :],
                in_=x_d[:, blk * RB : blk * RB + RB + 2, :],
            )
            if dup:
                # partitions 64:128 = same rows shifted one column left, so a
                # K=128 matmul contracts two horizontally-adjacent taps at
                # once.  Derived on-chip (cross-partition copy) instead of a
                # second DRAM read — the DMA system is the bottleneck.
                nc.vector.tensor_copy(
                    xb[C : 2 * C, :, 0 : WPAD - 1],
                    xb[0:C, :, 1:WPAD],
                )
            o_blk = outs.tile([OC, RB * W], o_dt)
            for s in range(SUB):
                r0 = s * ROWS_PER_TILE        # row offset within block
                if dup:
                    ps = psum.tile([128, NFREE], F32)
                    # 6 all-K=128 matmuls -> 3 col-tiled slots (K=128 streams
                    # ~2x faster per column than K=64 on this part, and the
                    # solo taps' bottom-half weights are zero):
                    #  A: pair(0,1) | pair(6,7) | solo k5
                    #  B: pair(3,4) | solo k2   | solo k8
                    mms = [
                        (0, 0, 0, True, False),   # pair row 0 -> A
                        (1, 1, 1, True, False),   # pair row 1 -> B
                        (2, 2, 0, False, False),  # pair row 2 -> A
                        (3, 0, 1, False, False),  # k2 (row 0, col 2) -> B
                        (4, 1, 0, False, True),   # k5 (row 1, col 2) -> A
                        (5, 2, 1, False, True),   # k8 (row 2, col 2) -> B
                    ]
                    for wi, i, col, st, sp in mms:
                        j = 0 if wi < 3 else 2
                        rhs = xb[:, r0 + i : r0 + i + ROWS_PER_TILE, j : j + W]
                        nc.tensor.matmul(
                            ps[64 * col : 64 * col + OC, :],
                            lhsT=w_sb[:, wi * OC : (wi + 1) * OC],
                            rhs=rhs,
                            start=st,
                            stop=sp,
                            tile_position=(0, 64 * col),
                            skip_group_check=True,
                        )
                elif tiled:
                    ps = psum.tile([128, NFREE], F32)
                    for k in range(9):
                        i, j = divmod(k, 3)
                        rhs = xb[:, r0 + i : r0 + i + ROWS_PER_TILE, j : j + W]
                        col = k % 2           # even taps -> A, odd -> B
                        nc.tensor.matmul(
                            ps[64 * col : 64 * col + OC, :],
                            lhsT=w_sb[:, k * OC : (k + 1) * OC],
                            rhs=rhs,
                            start=(k < 2),
                            stop=(k >= 7),
                            tile_position=(0, 64 * col),
                            skip_group_check=True,
                        )
                if tiled:
                    # Fold: B half evicted by ACT (with bias), A half added
                    # by DVE (walrus allows only one PSUM input per DVE op).
                    ps_a = ps[0:OC, 0:NFREE]
                    ps_b = ps[64:128, 0:NFREE]
                    tmpb = tmps.tile([OC, NFREE], F32)
                    nc.scalar.activation(
                        out=tmpb[:, :],
                        in_=ps_b,
                        func=mybir.ActivationFunctionType.Identity,
                        bias=b_sb[:, 0:1],
                        scale=1.0,
                    )
                    nc.vector.tensor_add(
                        o_blk[:, s * NFREE : (s + 1) * NFREE],
                        ps_a,
                        tmpb[:, :],
                    )
                else:
                    ps = psum.tile([OC, NFREE], F32)
                    for k in range(9):
                        i, j = divmod(k, 3)
                        rhs = xb[:, r0 + i : r0 + i + ROWS_PER_TILE, j : j + W]
                        nc.tensor.matmul(
                            ps[:, :],
                            lhsT=w_sb[:, k * OC : (k + 1) * OC],
                            rhs=rhs,
                            start=(k == 0),
                            stop=(k == 8),
                        )
                    nc.scalar.activation(
                        out=o_blk[:, s * NFREE : (s + 1) * NFREE],
                        in_=ps[:, :],
                        func=mybir.ActivationFunctionType.Identity,
                        bias=b_sb[:, 0:1],
                        scale=1.0,
                    )
            # Two half-block output DMAs: the first half drains while the
            # second half is still being computed (shorter kernel tail).
            HRB = RB // 2
            for h in range(2):
                nc.sync.dma_start(
                    out=o_d[:, blk * RB + h * HRB : blk * RB + (h + 1) * HRB, :],
                    in_=o_blk[:, h * HRB * W : (h + 1) * HRB * W].rearrange(
                        "p (r w) -> p r w", r=HRB
                    ),
                )
    if split_waits:
        _split_sync_waits(nc)
    return nc


def _split_sync_waits(nc, limit=1):
    """Hoist extra sync waits onto injected wait-only EventSemaphore ops.

    The neuronxcc walrus used under axon rejects compute instructions
    carrying more than one sync wait ("Too many sync wait commands", e.g.
    S3_LW / S3D3_AC structs).  Tile's sem assignment emits up to ~3.
    For every instruction with >limit waits, keep the first `limit` and
    prepend one wait-only EventSemaphore per extra wait on the same
    engine (same program position => same semantics).
    """
    import copy as _copy

    f = nc.m.functions[0]
    template = None
    for blk in f.blocks:
        for inst in blk.instructions:
            if type(inst).__name__ == "InstEventSemaphore":
                template = inst
                break
        if template is not None:
            break
    if template is None:
        return
    n_split = 0
    for blk in f.blocks:
        new_list = []
        changed = False
        for inst in blk.instructions:
            si = getattr(inst, "sync_info", None)
            op = str(getattr(inst, "opcode", ""))
            waits = list(si.on_wait) if (si and si.on_wait) else []
            if len(waits) > limit:
                for w in waits[limit:]:
                    ev = _copy.deepcopy(template)
                    ev.name = f"waitsplit_{n_split}"
                    n_split += 1
                    ev.engine = inst.engine
                    ev.sync_info = mybir.SyncInfo(on_wait=[w], on_update=[])
                    new_list.append(ev)
                inst.sync_info = mybir.SyncInfo(
                    on_wait=waits[:limit], on_update=list(si.on_update or [])
                )
                changed = True
            new_list.append(inst)
        if changed:
            blk.instructions = new_list


def _host_conv_weights(posi_map, w1, b1, w2, b2, w_ca, bias):
    """Collapse the constant-posi_map weight generator on the host."""
    pm = np.asarray(posi_map, np.float64)[0]              # [16, GP, GP]
    vvec = pm.reshape(POSI_CH, -1)[:, 0]                  # per-channel constant
    h = np.maximum(np.asarray(w1, np.float64) @ vvec + np.asarray(b1, np.float64), 0.0)
    wvec = np.asarray(w2, np.float64) @ h + np.asarray(b2, np.float64)   # [576]
    wk = wvec.reshape(OC, 9)                              # [o, k]
    wca = np.asarray(w_ca, np.float64)                    # [o, c]
    wfull = wk[:, None, :] * wca[:, :, None]              # [o, c, k]
    wts = np.ascontiguousarray(
        wfull.transpose(1, 2, 0).reshape(C, 9 * OC).astype(np.float32)
    )                                                     # [c, k*OC + o]
    wb = np.ascontiguousarray(
        np.asarray(bias, np.float32).reshape(OC, 1)
    )
    return wts, wb


def _pack_dup3(wts):
    """Repack [C, 9*OC] tap-major lhsT into the dup3 layout [128, 6*OC].

    Columns 0:3*OC are K=128 pairs (taps (3p, 3p+1) stacked on the
    partition axis, matching the +1-column-shifted input duplicate);
    columns 3*OC:6*OC are the K=64 solo taps (3q+2), bottom half zero.
    """
    w3 = np.zeros((128, 6 * OC), np.float32)
    for p in range(3):
        w3[0:C, p * OC:(p + 1) * OC] = wts[:, (3 * p) * OC:(3 * p + 1) * OC]
        w3[C:2 * C, p * OC:(p + 1) * OC] = wts[:, (3 * p + 1) * OC:(3 * p + 2) * OC]
        w3[0:C, (3 + p) * OC:(4 + p) * OC] = wts[:, (3 * p + 2) * OC:(3 * p + 3) * OC]
    return w3


def _numpy_reference(x, offset, posi_map, w1, b1, w2, b2, w_ca, bias):
    """Exact numpy port of reference.py (general-input fallback)."""
    x = np.asarray(x, np.float32)
    offset = np.asarray(offset, np.float32)
    posi_map = np.asarray(posi_map, np.float32)
    w1 = np.asarray(w1, np.float32)
    b1 = np.asarray(b1, np.float32)
    w2 = np.asarray(w2, np.float32)
    b2 = np.asarray(b2, np.float32)
    w_ca = np.asarray(w_ca, np.float32)
    bias = np.asarray(bias, np.float32)

    Bq, _, Hq, Wq = x.shape
    dx = offset[:, 0] * (2.0 / max(Wq - 1, 1)) * 0.5
    dy = offset[:, 1] * (2.0 / max(Hq - 1, 1)) * 0.5
    ys = np.linspace(-1.0, 1.0, Hq, dtype=x.dtype)
    xs = np.linspace(-1.0, 1.0, Wq, dtype=x.dtype)
    gx = xs[None, None, :] + dx
    gy = ys[None, :, None] + dy
    img = np.broadcast_to(posi_map, (Bq, posi_map.shape[1], GP, GP))

    Hp = Wp = GP
    imgT = img.transpose(0, 2, 3, 1)                      # [B, Hp, Wp, C]
    ix = np.clip((gx + 1.0) * 0.5 * (Wp - 1), 0.0, Wp - 1)
    iy = np.clip((gy + 1.0) * 0.5 * (Hp - 1), 0.0, Hp - 1)
    x0 = np.floor(ix).astype(np.int32)
    y0 = np.floor(iy).astype(np.int32)
    x1 = np.minimum(x0 + 1, Wp - 1)
    y1 = np.minimum(y0 + 1, Hp - 1)
    wx = (ix - x0.astype(ix.dtype))[..., None]
    wy = (iy - y0.astype(iy.dtype))[..., None]
    bb = np.arange(Bq)[:, None, None]
    v00 = imgT[bb, y0, x0]
    v01 = imgT[bb, y0, x1]
    v10 = imgT[bb, y1, x0]
    v11 = imgT[bb, y1, x1]
    top = v00 * (1 - wx) + v01 * wx
    bot = v10 * (1 - wx) + v11 * wx
    up = (top * (1 - wy) + bot * wy).transpose(0, 3, 1, 2)  # [B, 16, H, W]

    h = np.maximum(np.einsum('oc,bchw->bohw', w1, up) + b1[None, :, None, None], 0.0)
    weights = np.einsum('oc,bchw->bohw', w2, h) + b2[None, :, None, None]
    weights = weights.reshape(Bq, OC, KK * KK, Hq, Wq)
    x_adapt = np.einsum('oc,bchw->bohw', w_ca, x)
    xp = np.pad(x_adapt, ((0, 0), (0, 0), (1, 1), (1, 1)))
    patches = np.stack(
        [xp[:, :, i:i + Hq, j:j + Wq] for i in range(KK) for j in range(KK)],
        axis=2,
    )
    out = (weights * patches).sum(axis=2) + bias
    return out.astype(np.float32)


def kernel(**inputs):
    global _cached_nc, last_results
    x = np.ascontiguousarray(np.asarray(inputs["x"], np.float32))
    posi_map = np.asarray(inputs["posi_map"], np.float32)

    per_ch = posi_map.reshape(posi_map.shape[0] * posi_map.shape[1], -1)
    if not np.all(per_ch == per_ch[:, :1]):
        # general (spatially varying posi_map) fallback: exact numpy port
        return _numpy_reference(**{k: inputs[k] for k in (
            "x", "offset", "posi_map", "w1", "b1", "w2", "b2", "w_ca", "bias")})

    wts, wb = _host_conv_weights(
        posi_map, inputs["w1"], inputs["b1"], inputs["w2"], inputs["b2"],
        inputs["w_ca"], inputs["bias"],
    )

    variant = os.environ.get("BASS_KERNEL_VARIANT", "bf16_tiled")
    global _cached_variant
    if _cached_nc is None or _cached_variant != variant:
        _cached_nc = _build_conv_nc(variant)
        _cached_variant = variant

    xpad = np.pad(x, ((0, 0), (0, 0), (1, 1), (1, 1)))
    base_variant = variant[:-4] if variant.endswith("_o16") else variant
    if base_variant == "bf16_dup3":
        wts = _pack_dup3(wts)
    if base_variant in ("bf16_tiled", "bf16_dup3"):
        import ml_dtypes

        xpad = xpad.astype(ml_dtypes.bfloat16)
        wts = wts.astype(ml_dtypes.bfloat16)
    in_maps = [{"x": xpad[i], "wts": wts, "wb": wb} for i in range(NCORES)]
    trace = bool(int(os.environ.get("BASS_KERNEL_TRACE", "0")))
    if trace:
        _ensure_ntff_hook()
    res = run_bass_kernel_spmd(
        _cached_nc, in_maps, list(range(NCORES)), trace=trace
    )
    last_results = res
    out = np.stack(
        [np.asarray(res.results[i]["out"], np.float32) for i in range(NCORES)],
        axis=0,
    )
    return out
